# revision 11
# baseline (speedup 1.0000x reference)
"""DifferentiableLengthRegulator Trainium2 kernel.

out[b,c,l] = y_mask * (sum_t x[b,c,t]*W[b,t,l]) / (sum_t W[b,t,l] + eps)
W = exp(-0.5*(l - center[b,t])^2 / (w[b,t]^2*sigma_scale^2 + eps))

Sharding: data-parallel over batch B=16 -> 8 cores x 2 batches; batches are
clustered by cumsum midpoint so the SPMD band union across cores stays narrow.
Per core, per batch (banded over the frame axis; Gaussian weights vanish
outside ~5 sigma of each token chunk's centers):
  ACT : W = DerivErf(s*pos + (-s*center)) -> bf16  (per-partition scale+bias
        fold computes mu inside the ACT op: no DVE mu stage at all)
  PE  : psum[l, 0:257] += W_tc[:, lslice]^T @ [xT | ones]
  DVE : d+eps = tensor_scalar(psum cols 256); rd = 1/(d+eps);
        evac psum*rd -> bf16 (ACT takes some chunks as Copy-scale)
        (the eps is load-bearing: beyond a batch's total length all W
        underflow to 0, so 1/d alone would make inf*0 = NaN)
PSUM is one manually-rotated [CH, 8, 512] arena (8 banks): chunk j lives in
slot j%8, giving depth-8 rotation with per-range dependencies instead of
depth-2 tile-pool rotation (PE never waits on evac).
The PE clock is HAM-gated (1.2 GHz until ~3.4us of sustained busy): a block of
dummy matmuls on memset scratch warms it from t~6.7us so real matmuls run at
2.4 GHz. Startup DMAs are split across the scalar + sync HWDGE queues and xta
is loaded per token-chunk so nothing waits on a fat transfer.
Output layout [BPC, NT, CH, GRP, C] bf16 -> 2KB contiguous DMA lines; host
reshapes to (B, C, L) fp32. x_mask / y_mask folded on host.
"""

import numpy as np
import ml_dtypes

B, C, T, L = 16, 256, 512, 4096
N_CORES = 8
BPC = B // N_CORES  # batches per core
CH = 128            # partition chunk
TCN = T // CH       # 4 token chunks
LCN = L // CH       # 32 frame chunks
GRP = 4             # frame chunks per evac group
NT = LCN // GRP     # 8 groups per batch
NSLOT = 8           # psum arena slots (banks)
EPS = 1e-8
K_DERF = 1.1283791670955126  # 2/sqrt(pi), DerivErf's constant factor
MARGIN_SIGMA = 5.0
BAND_ALIGN = 128
NWARM = 13          # dummy matmuls to un-throttle the HAM clock gate
WARM_FREE = 192     # free dim of each dummy matmul
LOOKAHEAD = 4
POS_PIECES = (1024, 2304, 3584, 4096)  # iota piece ends (need-ordered)

_bf16 = ml_dtypes.bfloat16
_cache = {}

# DVE evac chunks per group (out of GRP=4); remainder goes to ACT as
# Copy-scale. Early groups run while ACT is still generating W -> all-DVE.
ND_PATTERN = {0: [4, 4, 4, 4, 3, 3, 3, 3], 1: [3] * 8}


def _center_scale(w, sigma_scale):
    """Mirror the reference's cumsum/center math (same jax backend bits)."""
    try:
        import jax.numpy as jnp

        wj = jnp.asarray(w)
        center = np.asarray(jnp.cumsum(wj, axis=1) - 0.5 * wj, dtype=np.float32)
    except Exception:
        center = (np.cumsum(w, axis=1, dtype=np.float32) - 0.5 * w).astype(np.float32)
    sigma = (w * np.float32(sigma_scale)).astype(np.float32)
    # W = DerivErf(s*mu)*sqrt(pi)/2 = exp(-(s*mu)^2), s = sqrt(0.5/(sig^2+eps))
    s = np.sqrt(np.float32(0.5) / (np.square(sigma) + np.float32(EPS))).astype(np.float32)
    return center, s


def _batch_order(center):
    """Cluster batches with similar center curves into the same SPMD slot so
    the per-slot band union across the 8 cores stays narrow."""
    return np.argsort(center[:, T // 2 - 1], kind="stable")


def _bands(center, w_all, order):
    """Per (slot, tc) aligned frame band, unioned across cores (SPMD)."""
    bands = []
    for slot in range(BPC):
        sel = order[slot * N_CORES:(slot + 1) * N_CORES]
        rows = center[sel]            # the 8 batches that land in this slot
        wrows = w_all[sel]
        sb = []
        for tc in range(TCN):
            seg = rows[:, tc * CH:(tc + 1) * CH]
            margin = float(MARGIN_SIGMA * wrows[:, tc * CH:(tc + 1) * CH].max() + 1.0)
            bs = max(0, int(np.floor((seg.min() - margin) / BAND_ALIGN)) * BAND_ALIGN)
            be = min(L, int(np.ceil((seg.max() + margin) / BAND_ALIGN)) * BAND_ALIGN)
            if tc == 0:
                bs = 0
            if tc == TCN - 1:
                be = L
            bs = min(bs, be - CH)
            sb.append((bs, be))
        bands.append(sb)
    for sb in bands:
        for chunk in range(LCN):
            lo = chunk * CH
            assert any(bs <= lo and lo + CH <= be for bs, be in sb), (
                f"frame chunk {chunk} uncovered; widen MARGIN_SIGMA"
            )
    return bands


def _split_excess_waits(nc, max_waits=1):
    """walrus here caps sync-waits at 1 per compute instruction; move the
    excess onto injected same-engine NoOps just before the instruction
    (waiting earlier on the same engine is always safe)."""
    from concourse import mybir

    for f in nc.m.functions:
        for blk in f.blocks:
            new = []
            for inst in blk.instructions:
                si = inst.sync_info
                if si is not None and len(si.on_wait) > max_waits:
                    waits = list(si.on_wait)
                    keep, extra = waits[-max_waits:], waits[:-max_waits]
                    for i in range(0, len(extra), max_waits):
                        nop = mybir.InstNoOp(name=f"{inst.name}-xw{i}", ins=[], outs=[])
                        nop.engine = inst.engine
                        nop.sync_info = mybir.SyncInfo(
                            on_wait=extra[i:i + max_waits], on_update=[])
                        new.append(nop)
                    inst.sync_info = mybir.SyncInfo(
                        on_wait=keep, on_update=list(si.on_update))
                new.append(inst)
            blk.instructions = new


def _slim_tile_exit(tile):
    """Drop the second all-engine barrier in Tile's exit sequence: the
    sem-clears it orders are already completed by each engine finishing its
    own instruction stream before the NEFF ends (~4us saved)."""
    if getattr(tile.TileContext, "_slim_exit", False):
        return
    ScopedClock = tile.ScopedClock

    def _drain_and_barrier(self, tick_clock, wait_clock):
        drain_inst = self.nc.sync.drain()
        wait_clock.add_sem_waits(
            drain_inst.ins, ScopedClock({None: tick_clock.global_clock}))
        self.nc.all_engine_barrier()
        popped = self.nc._tile_sem_poison_stack.pop()
        assert popped is self._sem_poison
        self.nc.clear_and_free_semaphores(list(self.sems.allocated().values()))

    tile.TileContext._drain_and_barrier = _drain_and_barrier
    tile.TileContext._slim_exit = True


def _build(band_key):
    import concourse.bass as bass
    import concourse.tile as tile
    from concourse import mybir

    _slim_tile_exit(tile)
    bands = [[(band_key[s][t][0], band_key[s][t][1]) for t in range(TCN)]
             for s in range(BPC)]
    wmax = [max(bands[s][t][1] - bands[s][t][0] for s in range(BPC))
            for t in range(TCN)]

    nc = bass.Bass("TRN2", target_bir_lowering=False, debug=False)
    # xta host layout: [b, p, tc, c] so the DMA is descriptor-light
    xta_d = nc.declare_dram_parameter("xta", [BPC, CH, TCN, C + 1], mybir.dt.bfloat16, isOutput=False)
    coefs_d = nc.declare_dram_parameter("coefs", [CH, 2 * BPC * TCN], mybir.dt.float32, isOutput=False)
    out_d = nc.declare_dram_parameter("out", [BPC, NT, CH, GRP, C], mybir.dt.bfloat16, isOutput=True)

    f32 = mybir.dt.float32
    bf16 = mybir.dt.bfloat16
    FT = mybir.ActivationFunctionType
    OP = mybir.AluOpType
    EPS_K = float(EPS) * K_DERF

    # first group (in the b0..b1 stream) whose matmuls need W(b, t)
    def first_need(b, t):
        bs = bands[b][t][0]
        return b * NT + bs // (GRP * CH)

    with tile.TileContext(nc) as tc_:
        import contextlib

        with contextlib.ExitStack() as ctx:
            consts = ctx.enter_context(tc_.tile_pool(name="consts", bufs=1))
            xta_p = ctx.enter_context(tc_.tile_pool(name="xta", bufs=2))
            w_pools = [ctx.enter_context(tc_.tile_pool(name=f"w{t}", bufs=2)) for t in range(TCN)]
            psum_p = ctx.enter_context(tc_.tile_pool(name="ps", bufs=1, space="PSUM"))
            small_p = ctx.enter_context(tc_.tile_pool(name="small", bufs=6))
            out_p = ctx.enter_context(tc_.tile_pool(name="osb", bufs=4))

            def col(tile_, idx):
                return tile_[:, idx:idx + 1]

            def cidx(q, b, t):
                # q=0 -> s (scale), q=1 -> -s*center (bias)
                return (q * BPC + b) * TCN + t

            # --- startup. Queue plan: scalar HWDGE carries xta(0,0) (its
            # table load hides the issue), sync carries coefs + the rest.
            xta_tiles = {b: xta_p.tile([CH, TCN, C + 1], bf16, tag="xta",
                                       name=f"xta{b}")
                         for b in range(BPC)}
            nc.scalar.dma_start(out=xta_tiles[0][:, 0, :], in_=xta_d[0, :, 0, :])

            coefs_sb = consts.tile([CH, 2 * BPC * TCN], f32)
            nc.sync.dma_start(out=coefs_sb, in_=coefs_d[:, :])
            for t in range(1, TCN):
                nc.sync.dma_start(out=xta_tiles[0][:, t, :], in_=xta_d[0, :, t, :])
            nc.sync.dma_start(out=xta_tiles[1], in_=xta_d[1])

            # HAM warm-up scratch: memset first so the dummy matmuls start
            # the moment the preamble ends (no DMA in their dep chain).
            warm1 = consts.tile([CH, 1], f32)
            nc.gpsimd.memset(warm1, 0.0)
            warm_sb = consts.tile([CH, 128 + WARM_FREE], bf16)
            nc.gpsimd.memset(warm_sb, 0.0)
            nc.scalar.activation(out=warm1, in_=warm1, func=FT.Derivative_Erf)

            # pos[l] = l from GpSimd iota, in need-ordered pieces; each W
            # tile's DERF is range-dep'd on just the pieces it reads.
            pos_f = consts.tile([CH, L], f32)
            lo = 0
            for hi in POS_PIECES:
                nc.gpsimd.iota(pos_f[:, lo:hi], pattern=[[1, hi - lo]], base=lo,
                               channel_multiplier=0,
                               allow_small_or_imprecise_dtypes=True)
                lo = hi

            w_tiles = {}

            def wgen(b, t, cuts=None):
                bs, be = bands[b][t]
                wt = w_pools[t].tile([CH, wmax[t]], bf16)
                edges = [bs] + sorted(c for c in set(cuts or ()) if bs < c < be) + [be]
                for lo, hi in zip(edges[:-1], edges[1:]):
                    # W = 2/sqrt(pi)*exp(-(s*pos - s*center)^2); the constant
                    # cancels via rd; scale+bias fold removes the mu stage.
                    nc.scalar.activation(
                        out=wt[:, lo - bs:hi - bs], in_=pos_f[:, lo:hi],
                        func=FT.Derivative_Erf,
                        scale=col(coefs_sb, cidx(0, b, t)),
                        bias=col(coefs_sb, cidx(1, b, t)),
                    )
                w_tiles[(b, t)] = wt

            # --- psum arena: one [CH, NSLOT, 512] allocation, manual
            # rotation chunk -> slot j%NSLOT; Tile range-deps do the rest.
            arena = psum_p.tile([CH, NSLOT, 512], f32)

            # dummy matmuls into the unused top of bank 7 (cols 257.. are
            # never touched by real chunks): PE busy from preamble end on,
            # so the HAM un-throttles to 2.4 GHz before real work lands.
            for _ in range(NWARM):
                nc.tensor.matmul(
                    out=arena[:, NSLOT - 1, 290:290 + WARM_FREE],
                    lhsT=warm_sb[:, :128],
                    rhs=warm_sb[:, 128:128 + WARM_FREE],
                    start=True, stop=True,
                )

            def chunk_matmuls(b, g, k):
                sb = bands[b]
                j0 = (g % (NSLOT // GRP)) * GRP
                lo = (g * GRP + k) * CH
                ctc = [t for t in range(TCN) if sb[t][0] <= lo and lo + CH <= sb[t][1]]
                for i, t in enumerate(ctc):
                    off = lo - sb[t][0]
                    nc.tensor.matmul(
                        out=arena[:, j0 + k, :C + 1],
                        lhsT=w_tiles[(b, t)][:, off:off + CH],
                        rhs=xta_tiles[b][:, t, :],
                        start=(i == 0), stop=(i == len(ctc) - 1),
                    )

            def emit_rd(slots_lo, slots_hi):
                """d+eps and reciprocal for psum slots [slots_lo, slots_hi).
                Batched across group pairs so the DVE pays the instruction
                overhead half as often, and emitted BEFORE the TTs it feeds
                so ACT copies never queue behind the DVE's TT backlog."""
                n = slots_hi - slots_lo
                dtmp = small_p.tile([CH, n], f32, tag="dtmp")
                nc.vector.tensor_scalar(
                    out=dtmp, in0=arena[:, slots_lo:slots_hi, C],
                    scalar1=EPS_K, scalar2=None, op0=OP.add,
                )
                rd = small_p.tile([CH, n], f32, tag="rd")
                nc.vector.reciprocal(out=rd, in_=dtmp)
                return rd, slots_lo

            def evac(b, g, nd, rd):
                rd_t, base = rd
                j0 = (g % (NSLOT // GRP)) * GRP
                stride = rd_t.ap[1][0]
                osb = out_p.tile([CH, GRP, C], bf16, tag="osb")
                rdb = bass.AP(tensor=rd_t.tensor,
                              offset=rd_t.offset + (j0 - base) * stride,
                              ap=[rd_t.ap[0], [stride, nd], [0, C]])
                nc.vector.tensor_tensor(
                    out=osb[:, :nd, :], in0=arena[:, j0:j0 + nd, :C], in1=rdb,
                    op=OP.mult)
                for k in range(nd, GRP):
                    nc.scalar.activation(
                        out=osb[:, k, :], in_=arena[:, j0 + k, :C],
                        func=FT.Copy, scale=col(rd_t, j0 - base + k))
                # late groups go on the (otherwise idle) scalar ring so the
                # sync ring's descriptor backlog is drained before the tail
                eng = nc.scalar if g in (NT - 4, NT - 3) and b == BPC - 1 else nc.sync
                eng.dma_start(out=out_d[b, g], in_=osb)

            def tail_group(b, g):
                # drain the final group in 2-chunk halves; the last half is
                # all-DVE (ACT is slower off psum and must not finish last)
                j0 = (g % (NSLOT // GRP)) * GRP
                osb = out_p.tile([CH, GRP, C], bf16, tag="osb")
                for k in (0, 1):
                    chunk_matmuls(b, g, k)
                rd, base = emit_rd(j0, j0 + 2)
                rdb = bass.AP(tensor=rd.tensor, offset=rd.offset,
                              ap=[rd.ap[0], [rd.ap[1][0], 1], [0, C]])
                nc.vector.tensor_tensor(
                    out=osb[:, 0, :], in0=arena[:, j0, :C], in1=rdb, op=OP.mult)
                nc.scalar.activation(
                    out=osb[:, 1, :], in_=arena[:, j0 + 1, :C],
                    func=FT.Copy, scale=col(rd, 1))
                nc.scalar.dma_start(out=out_d[b, g, :, 0:2, :], in_=osb[:, 0:2, :])
                for k in (2, 3):
                    chunk_matmuls(b, g, k)
                rd2, _ = emit_rd(j0 + 2, j0 + 4)
                rdb2 = bass.AP(tensor=rd2.tensor, offset=rd2.offset,
                               ap=[rd2.ap[0], [rd2.ap[1][0], 2], [0, C]])
                nc.vector.tensor_tensor(
                    out=osb[:, 2:4, :], in0=arena[:, j0 + 2:j0 + 4, :C],
                    in1=rdb2, op=OP.mult)
                nc.sync.dma_start(out=out_d[b, g, :, 2:4, :], in_=osb[:, 2:4, :])

            # --- schedule. Emission order per engine = execution order.
            # W tiles are emitted just-in-time, LOOKAHEAD groups before first
            # use; groups whose ACT copies would queue behind a DERF run
            # their whole evac on the DVE (nd=GRP). b=0 tiles get cuts at the
            # first group boundary inside their band (+ pos piece edges for
            # t=0) so the earliest matmuls unblock off a short DERF piece.
            def wcuts(b, t):
                if b != 0:
                    return None
                bs = bands[b][t][0]
                cuts = [((bs // (GRP * CH)) + 1) * GRP * CH]
                if t == 0:
                    cuts += [512, 1024]
                return cuts

            stream = [(b, g) for b in range(BPC) for g in range(NT)]
            need = sorted(
                ((first_need(b, t), b, t) for b in range(BPC) for t in range(TCN)),
            )
            emitted = set()

            def wgens_due(pos_idx):
                due = []
                for fn, b, t in need:
                    if fn <= pos_idx and (b, t) not in emitted:
                        emitted.add((b, t))
                        due.append((b, t))
                return due

            plan = []        # ('w', b, t) | ('g', b, g)
            for (b, t) in wgens_due(1):
                plan.append(('w', b, t))
            for idx, (b, g) in enumerate(stream):
                plan.append(('g', b, g))
                for (b2, t2) in wgens_due(idx + LOOKAHEAD):
                    plan.append(('w', b2, t2))

            def nd_of(b, g):
                idx = plan.index(('g', b, g))
                nxt = plan[idx + 1:idx + 2]
                return GRP if (nxt and nxt[0][0] == 'w') else ND_PATTERN[b][g]

            # Group pairing per batch: [0], [1,2], [3,4], [5,6], [7].
            # MMs for both pair members are emitted before the pair's rd so
            # Tile binds the rd's psum read to the right writers; the evac
            # then trails the PE by at most one group.
            for kind, x, y in plan:
                if kind == 'w':
                    wgen(x, y, cuts=wcuts(x, y))
                    continue
                b, g = x, y
                for k in range(GRP):
                    chunk_matmuls(b, g, k)
                j0 = (g % (NSLOT // GRP)) * GRP
                if g == 0:
                    evac(b, 0, nd_of(b, 0), emit_rd(0, GRP))
                elif g % 2 == 0:
                    rd = emit_rd(0, NSLOT)
                    evac(b, g - 1, nd_of(b, g - 1), rd)
                    evac(b, g, nd_of(b, g), rd)
                elif g == NT - 1:
                    if (b, g) == stream[-1]:
                        tail_group(b, g)
                    else:
                        evac(b, g, nd_of(b, g), emit_rd(j0, j0 + GRP))
    return nc


def _prepare_inputs(x, w, x_mask, y_mask, sigma_scale):
    center, s = _center_scale(w, sigma_scale[0])
    order = _batch_order(center)
    bands = _bands(center, w, order)
    nb = (-(s * center)).astype(np.float32)    # bias: -s*center (one f32 round)

    xm = np.broadcast_to(x_mask.reshape(B, T), (B, T)).astype(np.float32)
    if not np.all(xm == 1.0):
        x = (x * xm[:, None, :]).astype(np.float32)

    xt = np.ascontiguousarray(x.transpose(0, 2, 1))          # (B, T, C)
    xta = np.concatenate([xt, np.ones((B, T, 1), np.float32)], axis=2)
    # device layout [b, p, tc, c] for a descriptor-light DMA
    xta = np.ascontiguousarray(
        xta.reshape(B, TCN, CH, C + 1).transpose(0, 2, 1, 3)).astype(_bf16)

    in_maps = []
    for core in range(N_CORES):
        bsel = [int(order[s_ * N_CORES + core]) for s_ in range(BPC)]
        coefs = np.empty((2, BPC, TCN, CH), np.float32)
        for s_, bb in enumerate(bsel):
            coefs[0, s_] = s[bb].reshape(TCN, CH)
            coefs[1, s_] = nb[bb].reshape(TCN, CH)
        in_maps.append({
            "xta": xta[bsel],
            "coefs": np.ascontiguousarray(
                coefs.reshape(2 * BPC * TCN, CH).T),          # [CH, 16]
        })
    band_key = tuple(tuple(tuple(p) for p in sb) for sb in bands)
    return in_maps, band_key, order


def kernel(x, w, x_mask, y_mask, sigma_scale):
    x = np.asarray(x, dtype=np.float32)
    w = np.asarray(w, dtype=np.float32)
    x_mask = np.asarray(x_mask, dtype=np.float32)
    y_mask = np.asarray(y_mask, dtype=np.float32)
    sigma_scale = np.asarray(sigma_scale, dtype=np.float32)
    assert x.shape == (B, C, T) and w.shape == (B, T)

    in_maps, band_key, order = _prepare_inputs(x, w, x_mask, y_mask, sigma_scale)

    if band_key not in _cache:
        nc = _build(band_key)
        _split_excess_waits(nc)
        _cache[band_key] = nc
    nc = _cache[band_key]

    from concourse.bass_utils import run_bass_kernel_spmd

    res = run_bass_kernel_spmd(nc, in_maps, list(range(N_CORES)), trace=False)
    full = np.empty((B, L, C), np.float32)
    for core in range(N_CORES):
        o = np.asarray(res.results[core]["out"])             # (BPC, NT, CH, GRP, C)
        o = o.astype(np.float32).transpose(0, 1, 3, 2, 4).reshape(BPC, L, C)
        for s_ in range(BPC):
            full[int(order[s_ * N_CORES + core])] = o[s_]
    full = full.transpose(0, 2, 1)                           # (B, C, L)
    ym = np.broadcast_to(y_mask.reshape(B, L), (B, L)).astype(np.float32)
    if not np.all(ym == 1.0):
        full = full * ym[:, None, :]
    return full


# revision 14
# speedup vs baseline: 1.1747x; 1.1747x over previous
"""DifferentiableLengthRegulator Trainium2 kernel.

out[b,c,l] = y_mask * (sum_t x[b,c,t]*W[b,t,l]) / (sum_t W[b,t,l] + eps)
W = exp(-0.5*(l - center[b,t])^2 / (w[b,t]^2*sigma_scale^2 + eps))

Sharding: data-parallel over batch B=16 -> 8 cores x 2 batches; batches are
clustered by cumsum midpoint so the SPMD band union across cores stays narrow.
Per core, per batch (banded over the frame axis; Gaussian weights vanish
outside ~5 sigma of each token chunk's centers):
  ACT : W = DerivErf(s*pos + (-s*center)) -> bf16  (per-partition scale+bias
        fold computes mu inside the ACT op: no DVE mu stage at all)
  PE  : psum[l, 0:257] += W_tc[:, lslice]^T @ [xT | ones]
  DVE : d+eps = tensor_scalar(psum cols 256); rd = 1/(d+eps);
        evac psum*rd -> bf16 (ACT takes some chunks as Copy-scale)
        (the eps is load-bearing: beyond a batch's total length all W
        underflow to 0, so 1/d alone would make inf*0 = NaN)
PSUM is one manually-rotated [CH, 8, 512] arena (8 banks): chunk j lives in
slot j%8, giving depth-8 rotation with per-range dependencies instead of
depth-2 tile-pool rotation (PE never waits on evac).
The PE clock is HAM-gated (1.2 GHz until ~3.4us of sustained busy): a block of
dummy matmuls on memset scratch warms it from t~6.7us so real matmuls run at
2.4 GHz. Startup DMAs are split across the scalar + sync HWDGE queues and xta
is loaded per token-chunk so nothing waits on a fat transfer.
Output layout [BPC, NT, CH, GRP, C] bf16 -> 2KB contiguous DMA lines; host
reshapes to (B, C, L) fp32. x_mask / y_mask folded on host.
"""

import numpy as np
import ml_dtypes

B, C, T, L = 16, 256, 512, 4096
N_CORES = 8
BPC = B // N_CORES  # batches per core
CH = 128            # partition chunk
TCN = T // CH       # 4 token chunks
LCN = L // CH       # 32 frame chunks
GRP = 4             # frame chunks per evac group
NT = LCN // GRP     # 8 groups per batch
NSLOT = 8           # psum arena slots (banks)
EPS = 1e-8
K_DERF = 1.1283791670955126  # 2/sqrt(pi), DerivErf's constant factor
MARGIN_SIGMA = 5.0
BAND_ALIGN = 128
NWARM = 13          # dummy matmuls to un-throttle the HAM clock gate
WARM_FREE = 192     # free dim of each dummy matmul
LOOKAHEAD = 4
POS_PIECES = (1024, 2304, 3584, 4096)  # iota piece ends (need-ordered)

_bf16 = ml_dtypes.bfloat16
_cache = {}

# DVE evac chunks per group (out of GRP=4); remainder goes to ACT as
# Copy-scale. Early groups run while ACT is still generating W -> all-DVE.
ND_PATTERN = {0: [4, 4, 4, 4, 3, 3, 3, 3], 1: [3] * 8}


def _center_scale(w, sigma_scale):
    """Mirror the reference's cumsum/center math (same jax backend bits)."""
    try:
        import jax.numpy as jnp

        wj = jnp.asarray(w)
        center = np.asarray(jnp.cumsum(wj, axis=1) - 0.5 * wj, dtype=np.float32)
    except Exception:
        center = (np.cumsum(w, axis=1, dtype=np.float32) - 0.5 * w).astype(np.float32)
    sigma = (w * np.float32(sigma_scale)).astype(np.float32)
    # W = DerivErf(s*mu)*sqrt(pi)/2 = exp(-(s*mu)^2), s = sqrt(0.5/(sig^2+eps))
    s = np.sqrt(np.float32(0.5) / (np.square(sigma) + np.float32(EPS))).astype(np.float32)
    return center, s


def _batch_order(center):
    """Cluster batches with similar center curves into the same SPMD slot so
    the per-slot band union across the 8 cores stays narrow."""
    return np.argsort(center[:, T // 2 - 1], kind="stable")


def _bands(center, w_all, order):
    """Per (slot, tc) aligned frame band, unioned across cores (SPMD)."""
    bands = []
    for slot in range(BPC):
        sel = order[slot * N_CORES:(slot + 1) * N_CORES]
        rows = center[sel]            # the 8 batches that land in this slot
        wrows = w_all[sel]
        sb = []
        for tc in range(TCN):
            seg = rows[:, tc * CH:(tc + 1) * CH]
            margin = float(MARGIN_SIGMA * wrows[:, tc * CH:(tc + 1) * CH].max() + 1.0)
            bs = max(0, int(np.floor((seg.min() - margin) / BAND_ALIGN)) * BAND_ALIGN)
            be = min(L, int(np.ceil((seg.max() + margin) / BAND_ALIGN)) * BAND_ALIGN)
            if tc == 0:
                bs = 0
            if tc == TCN - 1:
                be = L
            bs = min(bs, be - CH)
            sb.append((bs, be))
        bands.append(sb)
    for sb in bands:
        for chunk in range(LCN):
            lo = chunk * CH
            assert any(bs <= lo and lo + CH <= be for bs, be in sb), (
                f"frame chunk {chunk} uncovered; widen MARGIN_SIGMA"
            )
    return bands


def _split_excess_waits(nc, max_waits=1):
    """walrus here caps sync-waits at 1 per compute instruction; move the
    excess onto injected same-engine NoOps just before the instruction
    (waiting earlier on the same engine is always safe)."""
    from concourse import mybir

    for f in nc.m.functions:
        for blk in f.blocks:
            new = []
            for inst in blk.instructions:
                si = inst.sync_info
                if si is not None and len(si.on_wait) > max_waits:
                    waits = list(si.on_wait)
                    keep, extra = waits[-max_waits:], waits[:-max_waits]
                    for i in range(0, len(extra), max_waits):
                        nop = mybir.InstNoOp(name=f"{inst.name}-xw{i}", ins=[], outs=[])
                        nop.engine = inst.engine
                        nop.sync_info = mybir.SyncInfo(
                            on_wait=extra[i:i + max_waits], on_update=[])
                        new.append(nop)
                    inst.sync_info = mybir.SyncInfo(
                        on_wait=keep, on_update=list(si.on_update))
                new.append(inst)
            blk.instructions = new


def _slim_tile_exit(tile):
    """Drop the second all-engine barrier in Tile's exit sequence: the
    sem-clears it orders are already completed by each engine finishing its
    own instruction stream before the NEFF ends (~4us saved)."""
    if getattr(tile.TileContext, "_slim_exit", False):
        return
    ScopedClock = tile.ScopedClock

    def _drain_and_barrier(self, tick_clock, wait_clock):
        drain_inst = self.nc.sync.drain()
        wait_clock.add_sem_waits(
            drain_inst.ins, ScopedClock({None: tick_clock.global_clock}))
        self.nc.all_engine_barrier()
        popped = self.nc._tile_sem_poison_stack.pop()
        assert popped is self._sem_poison
        self.nc.clear_and_free_semaphores(list(self.sems.allocated().values()))

    tile.TileContext._drain_and_barrier = _drain_and_barrier
    tile.TileContext._slim_exit = True


def _build(band_key):
    import concourse.bass as bass
    import concourse.tile as tile
    from concourse import mybir

    _slim_tile_exit(tile)
    bands = [[(band_key[s][t][0], band_key[s][t][1]) for t in range(TCN)]
             for s in range(BPC)]
    wmax = [max(bands[s][t][1] - bands[s][t][0] for s in range(BPC))
            for t in range(TCN)]

    nc = bass.Bass("TRN2", target_bir_lowering=False, debug=False)
    # xta host layout: [b, p, tc, c] so the DMA is descriptor-light
    xta_d = nc.declare_dram_parameter("xta", [BPC, CH, TCN, C + 1], mybir.dt.bfloat16, isOutput=False)
    coefs_d = nc.declare_dram_parameter("coefs", [CH, 2 * BPC * TCN], mybir.dt.float32, isOutput=False)
    out_d = nc.declare_dram_parameter("out", [BPC, NT, CH, GRP, C], mybir.dt.bfloat16, isOutput=True)

    f32 = mybir.dt.float32
    bf16 = mybir.dt.bfloat16
    FT = mybir.ActivationFunctionType
    OP = mybir.AluOpType
    EPS_K = float(EPS) * K_DERF

    # first group (in the b0..b1 stream) whose matmuls need W(b, t)
    def first_need(b, t):
        bs = bands[b][t][0]
        return b * NT + bs // (GRP * CH)

    with tile.TileContext(nc) as tc_:
        import contextlib

        with contextlib.ExitStack() as ctx:
            consts = ctx.enter_context(tc_.tile_pool(name="consts", bufs=1))
            xta_p = ctx.enter_context(tc_.tile_pool(name="xta", bufs=2))
            w_pools = [ctx.enter_context(tc_.tile_pool(name=f"w{t}", bufs=2)) for t in range(TCN)]
            psum_p = ctx.enter_context(tc_.tile_pool(name="ps", bufs=1, space="PSUM"))
            small_p = ctx.enter_context(tc_.tile_pool(name="small", bufs=6))
            out_p = ctx.enter_context(tc_.tile_pool(name="osb", bufs=4))

            def col(tile_, idx):
                return tile_[:, idx:idx + 1]

            def cidx(q, b, t):
                # q=0 -> s (scale), q=1 -> -s*center (bias)
                return (q * BPC + b) * TCN + t

            # --- startup. Queue plan: scalar HWDGE carries xta(0,0) (its
            # table load hides the issue), sync carries coefs + the rest.
            xta_tiles = {b: xta_p.tile([CH, TCN, C + 1], bf16, tag="xta",
                                       name=f"xta{b}")
                         for b in range(BPC)}
            nc.scalar.dma_start(out=xta_tiles[0][:, 0, :], in_=xta_d[0, :, 0, :])

            coefs_sb = consts.tile([CH, 2 * BPC * TCN], f32)
            nc.sync.dma_start(out=coefs_sb, in_=coefs_d[:, :])
            for t in range(1, TCN):
                nc.sync.dma_start(out=xta_tiles[0][:, t, :], in_=xta_d[0, :, t, :])
            nc.sync.dma_start(out=xta_tiles[1], in_=xta_d[1])

            # HAM warm-up scratch: memset first so the dummy matmuls start
            # the moment the preamble ends (no DMA in their dep chain).
            warm_sb = consts.tile([CH, 128 + WARM_FREE], bf16)
            nc.gpsimd.memset(warm_sb, 0.0)
            warm1 = consts.tile([CH, 1], f32)
            nc.gpsimd.memset(warm1, 0.0)
            nc.scalar.activation(out=warm1, in_=warm1, func=FT.Derivative_Erf)

            # pos[l] = l from GpSimd iota, in need-ordered pieces; each W
            # tile's DERF is range-dep'd on just the pieces it reads.
            pos_f = consts.tile([CH, L], f32)
            lo = 0
            for hi in POS_PIECES:
                nc.gpsimd.iota(pos_f[:, lo:hi], pattern=[[1, hi - lo]], base=lo,
                               channel_multiplier=0,
                               allow_small_or_imprecise_dtypes=True)
                lo = hi

            w_tiles = {}

            def wgen(b, t, cuts=None):
                bs, be = bands[b][t]
                wt = w_pools[t].tile([CH, wmax[t]], bf16)
                edges = [bs] + sorted(c for c in set(cuts or ()) if bs < c < be) + [be]
                for lo, hi in zip(edges[:-1], edges[1:]):
                    # W = 2/sqrt(pi)*exp(-(s*pos - s*center)^2); the constant
                    # cancels via rd; scale+bias fold removes the mu stage.
                    nc.scalar.activation(
                        out=wt[:, lo - bs:hi - bs], in_=pos_f[:, lo:hi],
                        func=FT.Derivative_Erf,
                        scale=col(coefs_sb, cidx(0, b, t)),
                        bias=col(coefs_sb, cidx(1, b, t)),
                    )
                w_tiles[(b, t)] = wt

            # --- psum arena: one [CH, NSLOT, 512] allocation, manual
            # rotation chunk -> slot j%NSLOT; Tile range-deps do the rest.
            arena = psum_p.tile([CH, NSLOT, 512], f32)

            # dummy matmuls into the unused top of bank 7 (cols 257.. are
            # never touched by real chunks): PE busy from preamble end on,
            # so the HAM un-throttles to 2.4 GHz before real work lands.
            for _ in range(NWARM):
                nc.tensor.matmul(
                    out=arena[:, NSLOT - 1, 290:290 + WARM_FREE],
                    lhsT=warm_sb[:, :128],
                    rhs=warm_sb[:, 128:128 + WARM_FREE],
                    start=True, stop=True,
                )

            def chunk_matmuls(b, g, k):
                sb = bands[b]
                j0 = (g % (NSLOT // GRP)) * GRP
                lo = (g * GRP + k) * CH
                ctc = [t for t in range(TCN) if sb[t][0] <= lo and lo + CH <= sb[t][1]]
                for i, t in enumerate(ctc):
                    off = lo - sb[t][0]
                    nc.tensor.matmul(
                        out=arena[:, j0 + k, :C + 1],
                        lhsT=w_tiles[(b, t)][:, off:off + CH],
                        rhs=xta_tiles[b][:, t, :],
                        start=(i == 0), stop=(i == len(ctc) - 1),
                    )

            def emit_rd(slots_lo, slots_hi):
                """d+eps and reciprocal for psum slots [slots_lo, slots_hi).
                Batched across group pairs so the DVE pays the instruction
                overhead half as often, and emitted BEFORE the TTs it feeds
                so ACT copies never queue behind the DVE's TT backlog."""
                n = slots_hi - slots_lo
                dtmp = small_p.tile([CH, n], f32, tag="dtmp")
                nc.vector.tensor_scalar(
                    out=dtmp, in0=arena[:, slots_lo:slots_hi, C],
                    scalar1=EPS_K, scalar2=None, op0=OP.add,
                )
                rd = small_p.tile([CH, n], f32, tag="rd")
                nc.vector.reciprocal(out=rd, in_=dtmp)
                return rd, slots_lo

            def evac(b, g, nd, rd):
                rd_t, base = rd
                j0 = (g % (NSLOT // GRP)) * GRP
                stride = rd_t.ap[1][0]
                osb = out_p.tile([CH, GRP, C], bf16, tag="osb")
                rdb = bass.AP(tensor=rd_t.tensor,
                              offset=rd_t.offset + (j0 - base) * stride,
                              ap=[rd_t.ap[0], [stride, nd], [0, C]])
                nc.vector.tensor_tensor(
                    out=osb[:, :nd, :], in0=arena[:, j0:j0 + nd, :C], in1=rdb,
                    op=OP.mult)
                for k in range(nd, GRP):
                    nc.scalar.activation(
                        out=osb[:, k, :], in_=arena[:, j0 + k, :C],
                        func=FT.Copy, scale=col(rd_t, j0 - base + k))
                # late groups go on the (otherwise idle) scalar ring so the
                # sync ring's descriptor backlog is drained before the tail
                eng = nc.scalar if g in (NT - 4, NT - 3) and b == BPC - 1 else nc.sync
                eng.dma_start(out=out_d[b, g], in_=osb)

            def tail_group(b, g, rd):
                # final group: DVE sweeps chunks 0-2 -> sync ring while ACT
                # takes chunk 3 -> scalar ring in parallel; both rings are
                # drained by now so the two DMAs issue immediately.
                rd_t, base = rd
                j0 = (g % (NSLOT // GRP)) * GRP
                stride = rd_t.ap[1][0]
                osb = out_p.tile([CH, GRP, C], bf16, tag="osb")
                nc.scalar.activation(
                    out=osb[:, 3, :], in_=arena[:, j0 + 3, :C],
                    func=FT.Copy, scale=col(rd_t, j0 - base + 3))
                nc.scalar.dma_start(out=out_d[b, g, :, 3:4, :], in_=osb[:, 3:4, :])
                rdb = bass.AP(tensor=rd_t.tensor,
                              offset=rd_t.offset + (j0 - base) * stride,
                              ap=[rd_t.ap[0], [stride, 3], [0, C]])
                nc.vector.tensor_tensor(
                    out=osb[:, 0:3, :], in0=arena[:, j0:j0 + 3, :C], in1=rdb,
                    op=OP.mult)
                nc.sync.dma_start(out=out_d[b, g, :, 0:3, :], in_=osb[:, 0:3, :])

            # --- schedule. Emission order per engine = execution order.
            # W tiles are emitted just-in-time, LOOKAHEAD groups before first
            # use; groups whose ACT copies would queue behind a DERF run
            # their whole evac on the DVE (nd=GRP). b=0 tiles get cuts at the
            # first group boundary inside their band (+ pos piece edges for
            # t=0) so the earliest matmuls unblock off a short DERF piece.
            def wcuts(b, t):
                if b != 0:
                    return None
                bs = bands[b][t][0]
                cuts = [((bs // (GRP * CH)) + 1) * GRP * CH]
                if t == 0:
                    cuts += [512, 1024]
                return cuts

            stream = [(b, g) for b in range(BPC) for g in range(NT)]
            need = sorted(
                ((first_need(b, t), b, t) for b in range(BPC) for t in range(TCN)),
            )
            emitted = set()

            def wgens_due(pos_idx):
                due = []
                for fn, b, t in need:
                    if fn <= pos_idx and (b, t) not in emitted:
                        emitted.add((b, t))
                        due.append((b, t))
                return due

            plan = []        # ('w', b, t) | ('g', b, g)
            for (b, t) in wgens_due(1):
                plan.append(('w', b, t))
            for idx, (b, g) in enumerate(stream):
                plan.append(('g', b, g))
                for (b2, t2) in wgens_due(idx + LOOKAHEAD):
                    plan.append(('w', b2, t2))

            def nd_of(b, g):
                idx = plan.index(('g', b, g))
                nxt = plan[idx + 1:idx + 2]
                return GRP if (nxt and nxt[0][0] == 'w') else ND_PATTERN[b][g]

            # Per group: MMs then its rd immediately (DVE), but the evac of
            # the PREVIOUS group. The one-group evac lag means every TT and
            # every ACT copy finds its rd long since computed — ACT never
            # queues behind the DVE's TT backlog — while psum slots still
            # free at the same cadence as the PE consumes them.
            pending = None
            for kind, x, y in plan:
                if kind == 'w':
                    wgen(x, y, cuts=wcuts(x, y))
                    continue
                b, g = x, y
                for k in range(GRP):
                    chunk_matmuls(b, g, k)
                j0 = (g % (NSLOT // GRP)) * GRP
                rd = emit_rd(j0, j0 + GRP)
                if pending is not None:
                    pb, pg, prd = pending
                    evac(pb, pg, nd_of(pb, pg), prd)
                pending = (b, g, rd)
            tail_group(*pending[:2], pending[2])
    return nc


def _prepare_inputs(x, w, x_mask, y_mask, sigma_scale):
    center, s = _center_scale(w, sigma_scale[0])
    order = _batch_order(center)
    bands = _bands(center, w, order)
    nb = (-(s * center)).astype(np.float32)    # bias: -s*center (one f32 round)

    xm = np.broadcast_to(x_mask.reshape(B, T), (B, T)).astype(np.float32)
    if not np.all(xm == 1.0):
        x = (x * xm[:, None, :]).astype(np.float32)

    xt = np.ascontiguousarray(x.transpose(0, 2, 1))          # (B, T, C)
    xta = np.concatenate([xt, np.ones((B, T, 1), np.float32)], axis=2)
    # device layout [b, p, tc, c] for a descriptor-light DMA
    xta = np.ascontiguousarray(
        xta.reshape(B, TCN, CH, C + 1).transpose(0, 2, 1, 3)).astype(_bf16)

    in_maps = []
    for core in range(N_CORES):
        bsel = [int(order[s_ * N_CORES + core]) for s_ in range(BPC)]
        coefs = np.empty((2, BPC, TCN, CH), np.float32)
        for s_, bb in enumerate(bsel):
            coefs[0, s_] = s[bb].reshape(TCN, CH)
            coefs[1, s_] = nb[bb].reshape(TCN, CH)
        in_maps.append({
            "xta": xta[bsel],
            "coefs": np.ascontiguousarray(
                coefs.reshape(2 * BPC * TCN, CH).T),          # [CH, 16]
        })
    band_key = tuple(tuple(tuple(p) for p in sb) for sb in bands)
    return in_maps, band_key, order


def kernel(x, w, x_mask, y_mask, sigma_scale):
    x = np.asarray(x, dtype=np.float32)
    w = np.asarray(w, dtype=np.float32)
    x_mask = np.asarray(x_mask, dtype=np.float32)
    y_mask = np.asarray(y_mask, dtype=np.float32)
    sigma_scale = np.asarray(sigma_scale, dtype=np.float32)
    assert x.shape == (B, C, T) and w.shape == (B, T)

    in_maps, band_key, order = _prepare_inputs(x, w, x_mask, y_mask, sigma_scale)

    if band_key not in _cache:
        nc = _build(band_key)
        _split_excess_waits(nc)
        _cache[band_key] = nc
    nc = _cache[band_key]

    from concourse.bass_utils import run_bass_kernel_spmd

    res = run_bass_kernel_spmd(nc, in_maps, list(range(N_CORES)), trace=False)
    full = np.empty((B, L, C), np.float32)
    for core in range(N_CORES):
        o = np.asarray(res.results[core]["out"])             # (BPC, NT, CH, GRP, C)
        o = o.astype(np.float32).transpose(0, 1, 3, 2, 4).reshape(BPC, L, C)
        for s_ in range(BPC):
            full[int(order[s_ * N_CORES + core])] = o[s_]
    full = full.transpose(0, 2, 1)                           # (B, C, L)
    ym = np.broadcast_to(y_mask.reshape(B, L), (B, L)).astype(np.float32)
    if not np.all(ym == 1.0):
        full = full * ym[:, None, :]
    return full
